# revision 8
# baseline (speedup 1.0000x reference)
"""Trainium2 Bass kernel for a 12-layer BERT-style encoder with star-token embedding
lookup (KVPLM). Full inputs in, full outputs out; data-parallel over batch across
8 NeuronCores (B=16 -> 2 per core), weights replicated.

Per-core design (per batch b of 2, tokens 512 = 4 blocks x 128 partitions):
  - activations kept SBUF-resident in "natural" [tok_p, feat] layout; a transposed
    f32r copy [feat_p, tok] is produced by PE transposes where matmuls need it
  - all matmuls run in float32r (fp32 with 11-bit mantissa) at full PE rate
  - attention computed with transposed scores S^T[j,i] = K^T.T @ Q^T so softmax
    normalization folds into a ones-column of the V stationary (denominator comes
    out as row 64 of the ctx PSUM) + a K=1 PE broadcast of the reciprocal
  - embedding gather via indirect DMA from a host-concatenated [word;star] table
  - LayerNorm via bn_stats/bn_aggr; GELU/exp/tanh on the scalar engine

kernel(**inputs) -> (x [16,512,768] f32, pooled [16,768] f32)
"""
import os
import numpy as np

# model dims (hardcoded per problem spec)
L, H, NH, F, S, VOCAB, STAR_V, OFF = 12, 768, 12, 3072, 512, 31090, 390, 30700
DH = H // NH              # 64
B = 16
NCORES = 8
BP = B // NCORES          # batches per core = 2
P = 128
TB = S // P               # token blocks per batch = 4
NT = BP * TB              # token tiles per core = 8
KO = H // P               # feature blocks = 6
ICH = F // 512            # FFN inter chunks of 512 = 6
KC = 512 // P             # k-blocks per chunk = 4
EPS = 1e-12
SCALE = 1.0 / 8.0         # 1/sqrt(64)

_CACHE: dict = {}
LAST: dict = {}


def _build(use_ext: bool, n_layers: int, taps: bool = False):
    import concourse.bass as bass
    import concourse.tile as tile
    import concourse.mybir as mybir
    from concourse import bacc
    from concourse.masks import make_identity

    F32, F32R, I32 = mybir.dt.float32, mybir.dt.float32r, mybir.dt.int32
    AF = mybir.ActivationFunctionType
    OP = mybir.AluOpType

    nc = bacc.Bacc("TRN2", target_bir_lowering=False, debug=False)

    # ---------------- DRAM I/O ----------------
    emb = nc.dram_tensor("emb", [VOCAB + STAR_V, H], F32, kind="ExternalInput").ap()
    tok_idx = nc.dram_tensor("tok_idx", [NT * P, 1], I32, kind="ExternalInput").ap()
    pt_dram = nc.dram_tensor("pt", [BP * S, H], F32, kind="ExternalInput").ap()
    wq_d = nc.dram_tensor("wq", [L, H, H], F32, kind="ExternalInput").ap()
    wk_d = nc.dram_tensor("wk", [L, H, H], F32, kind="ExternalInput").ap()
    wv_d = nc.dram_tensor("wv", [L, H, H], F32, kind="ExternalInput").ap()
    wo_d = nc.dram_tensor("wo", [L, H, H], F32, kind="ExternalInput").ap()
    wi_d = nc.dram_tensor("wi", [L, H, F], F32, kind="ExternalInput").ap()
    wf_d = nc.dram_tensor("wf", [L, F, H], F32, kind="ExternalInput").ap()
    pw_d = nc.dram_tensor("pw", [H, H], F32, kind="ExternalInput").ap()
    if use_ext:
        ext_d = nc.dram_tensor("ext", [BP * S, 1], F32, kind="ExternalInput").ap()

    x_out = nc.dram_tensor("x_out", [BP * S, H], F32, kind="ExternalOutput").ap()
    pooled_out = nc.dram_tensor("pooled_out", [BP, H], F32, kind="ExternalOutput").ap()
    bounce = nc.dram_tensor("bounce", [BP, H], F32).ap()  # internal scratch
    tap = {}
    if taps:
        for nm, shp in [("xT", [P, KO, S]), ("QT", [P, KO, S]), ("KT", [P, KO, S]),
                        ("vaug", [P, TB, NH, DH + 1]), ("e0", [P, TB, S]),
                        ("ctxT", [P, KO, S]), ("den0", [1, S]),
                        ("xs", [P, TB, H]), ("x2", [P, TB, H]),
                        ("hT0", [P, KC, S]), ("xs2", [P, TB, H])]:
            tap[nm] = nc.dram_tensor("tap_" + nm, shp, F32,
                                     kind="ExternalOutput").ap()

    with tile.TileContext(nc) as tc:
        with (
            tc.tile_pool(name="const", bufs=1) as const,
            tc.tile_pool(name="xres", bufs=3) as xres,
            tc.tile_pool(name="work", bufs=3) as work,
            tc.tile_pool(name="tT", bufs=4) as tT,
            tc.tile_pool(name="hTp", bufs=2) as hTp,
            tc.tile_pool(name="ep", bufs=2) as ep,
            tc.tile_pool(name="w4", bufs=2) as w4,
            tc.tile_pool(name="wif", bufs=2) as wif,
            tc.tile_pool(name="st", bufs=4) as st,
            tc.tile_pool(name="rb", bufs=2) as rb,
            tc.tile_pool(name="pl", bufs=1) as pl,
            tc.tile_pool(name="mm", bufs=2, space="PSUM") as mm,
            tc.tile_pool(name="ctxp", bufs=2, space="PSUM") as ctxp,
            tc.tile_pool(name="accp", bufs=4, space="PSUM") as accp,
        ):
            ident = const.tile([P, P], F32)
            make_identity(nc, ident)
            eps_sb = const.tile([P, 1], F32)
            nc.vector.memset(eps_sb, EPS)
            ones_f32 = const.tile([P, TB, NH, 1], F32)
            nc.vector.memset(ones_f32, 1.0)
            ones64_f32 = const.tile([1, DH], F32)
            nc.vector.memset(ones64_f32, 1.0)
            ones_col = const.tile([1, DH], F32R)
            nc.vector.tensor_copy(ones_col, ones64_f32)
            # V augmented with a ones column per head: [j_p, jb, head, dh+1]
            v_aug = const.tile([P, TB, NH, DH + 1], F32R)
            nc.vector.tensor_copy(v_aug[:, :, :, DH:DH + 1], ones_f32)
            idx_sb = const.tile([P, NT], I32)
            nc.sync.dma_start(idx_sb, tok_idx.rearrange("(t p) o -> p (t o)", p=P))
            if use_ext:
                ext_sb = const.tile([P, BP, TB, 1], F32)
                nc.sync.dma_start(
                    ext_sb, ext_d.rearrange("(b t p) o -> p b t (o)", p=P, b=BP))

            # ---------------- helpers ----------------
            def layer_norm(xin, outpool):
                """xin: tile [P, TB, H] f32 -> new tile from outpool, LN over H."""
                out = outpool.tile([P, TB, H], F32, tag="x")
                for tb in range(TB):
                    stats = st.tile([P, 3, 6], F32, tag="st")
                    for sg in range(3):
                        nc.vector.bn_stats(stats[:, sg, :],
                                           xin[:, tb, sg * 256:(sg + 1) * 256])
                    mv = st.tile([P, 2], F32, tag="mv")
                    nc.vector.bn_aggr(mv, stats)
                    std = st.tile([P, 1], F32, tag="sd")
                    nc.scalar.activation(std, mv[:, 1:2], AF.Sqrt, bias=eps_sb[:, :1])
                    rstd = st.tile([P, 1], F32, tag="rs")
                    nc.vector.reciprocal(rstd, std)
                    nc.vector.tensor_scalar(
                        out=out[:, tb, :], in0=xin[:, tb, :],
                        scalar1=mv[:, 0:1], scalar2=rstd[:, :1],
                        op0=OP.subtract, op1=OP.mult)
                return out

            def transpose_b(xin):
                """xin: tile [P, TB, H] f32 -> f32r [P(feat), KO, S] transposed."""
                xT = tT.tile([P, KO, S], F32R, tag="T")
                for fo in range(KO):
                    for tb in range(TB):
                        ps = mm.tile([P, 512], F32, tag="mm")
                        nc.tensor.transpose(ps[:, :P], xin[:, tb, fo * P:(fo + 1) * P],
                                            ident)
                        nc.vector.tensor_copy(xT[:, fo, tb * P:(tb + 1) * P],
                                              ps[:, :P])
                return xT

            def load_w4(dram_l):  # [H, H] dram slice -> 2 chunk tiles [P, KO, 384] f32r
                tiles = []
                for ch in range(2):
                    t = w4.tile([P, KO, 384], F32R, tag="w4")
                    nc.sync.dma_start(
                        t, dram_l[:, ch * 384:(ch + 1) * 384]
                        .rearrange("(ko p) n -> p ko n", p=P).bitcast(F32R))
                    tiles.append(t)
                return tiles

            # ---------------- embedding ----------------
            x_b = []
            for b in range(BP):
                xe = work.tile([P, TB, H], F32, tag="x")
                for tb in range(TB):
                    nc.gpsimd.indirect_dma_start(
                        out=xe[:, tb, :], out_offset=None, in_=emb,
                        in_offset=bass.IndirectOffsetOnAxis(
                            ap=idx_sb[:, b * TB + tb:b * TB + tb + 1], axis=0))
                ptb = work.tile([P, TB, H], F32, tag="x")
                nc.sync.dma_start(
                    ptb, pt_dram[b * S:(b + 1) * S].rearrange("(t p) f -> p t f", p=P))
                xs = work.tile([P, TB, H], F32, tag="x")
                nc.vector.tensor_tensor(out=xs, in0=xe, in1=ptb, op=OP.add)
                x_b.append(layer_norm(xs, xres))

            # ---------------- encoder layers ----------------
            for l in range(n_layers):
                for b in range(BP):
                    xT = transpose_b(x_b[b])

                    # Q^T, K^T: [feat_out_p, KO, S]; stationary = W chunks
                    qkT = []
                    for w_dram in (wq_d, wk_d):
                        w_c = load_w4(w_dram[l])
                        t = tT.tile([P, KO, S], F32R, tag="T")
                        for ch in range(2):
                            for hl in range(3):
                                ps = mm.tile([P, 512], F32, tag="mm")
                                for k in range(KO):
                                    nc.tensor.matmul(
                                        ps, lhsT=w_c[ch][:, k, hl * P:(hl + 1) * P],
                                        rhs=xT[:, k, :],
                                        start=(k == 0), stop=(k == KO - 1))
                                nc.vector.tensor_copy(t[:, ch * 3 + hl, :], ps)
                        qkT.append(t)
                    QT, KT = qkT
                    if taps and l == 0 and b == 0:
                        nc.sync.dma_start(tap["xT"], xT.bitcast(F32))
                        nc.sync.dma_start(tap["QT"], QT.bitcast(F32))
                        nc.sync.dma_start(tap["KT"], KT.bitcast(F32))

                    # V natural into v_aug[:, tb, h, :DH]
                    wv_c = load_w4(wv_d[l])
                    for ch in range(2):
                        for tb in range(TB):
                            ps = mm.tile([P, 512], F32, tag="mm")
                            for k in range(KO):
                                nc.tensor.matmul(
                                    ps[:, :384], lhsT=xT[:, k, tb * P:(tb + 1) * P],
                                    rhs=wv_c[ch][:, k, :],
                                    start=(k == 0), stop=(k == KO - 1))
                            nc.vector.tensor_copy(
                                v_aug[:, tb, ch * 6:(ch + 1) * 6, 0:DH],
                                ps[:, :384].rearrange("p (h d) -> p h d", d=DH))

                    if taps and l == 0 and b == 0:
                        nc.sync.dma_start(tap["vaug"], v_aug.bitcast(F32))

                    # attention per head
                    ctxT = tT.tile([P, KO, S], F32R, tag="T")
                    for h in range(NH):
                        pb = DH * (h % 2)
                        fo = h // 2
                        qT_h = QT[pb:pb + DH, fo, :]
                        ctx_ps = ctxp.tile([P, 512], F32, tag="ctx")
                        for jb in range(TB):
                            s_ps = mm.tile([P, 512], F32, tag="mm")
                            nc.tensor.matmul(
                                s_ps, lhsT=KT[pb:pb + DH, fo, jb * P:(jb + 1) * P],
                                rhs=qT_h, start=True, stop=True)
                            e_t = ep.tile([P, 512], F32R, tag="e")
                            bias = ext_sb[:, b, jb, :1] if use_ext else 0.0
                            nc.scalar.activation(e_t, s_ps, AF.Exp,
                                                 scale=SCALE, bias=bias)
                            if taps and l == 0 and b == 0 and h == 0:
                                nc.sync.dma_start(tap["e0"][:, jb, :],
                                                  e_t.bitcast(F32))
                            nc.tensor.matmul(
                                ctx_ps[0:DH + 1, :], lhsT=v_aug[:, jb, h, :],
                                rhs=e_t, start=(jb == 0), stop=(jb == TB - 1))
                        # normalize: ctxT[head rows] = ctx * (1/denom) broadcast
                        rec = rb.tile([1, 512], F32R, tag="rec")
                        with nc.allow_low_precision(reason="f32r recip"):
                            nc.vector.reciprocal(rec, ctx_ps[DH:DH + 1, :])
                        bc_ps = ctxp.tile([P, 512], F32, tag="ctx")
                        nc.tensor.matmul(bc_ps[0:DH, :], lhsT=ones_col, rhs=rec,
                                         start=True, stop=True)
                        bc_sb = rb.tile([DH, 512], F32, tag="bc")
                        nc.vector.tensor_copy(bc_sb, bc_ps[0:DH, :])
                        if taps and l == 0 and b == 0 and h == 0:
                            nc.sync.dma_start(tap["den0"], rec.bitcast(F32))
                        nc.vector.tensor_tensor(
                            out=ctxT[pb:pb + DH, fo, :], in0=ctx_ps[0:DH, :],
                            in1=bc_sb, op=OP.mult)

                    # O projection + residual
                    wo_c = load_w4(wo_d[l])
                    xs = work.tile([P, TB, H], F32, tag="x")
                    for tb in range(TB):
                        for ch in range(2):
                            acc = accp.tile([P, 384], F32, tag="acc")
                            for k in range(KO):
                                nc.tensor.matmul(
                                    acc,
                                    lhsT=ctxT[:, k, tb * P:(tb + 1) * P],
                                    rhs=wo_c[ch][:, k, :],
                                    start=(k == 0), stop=(k == KO - 1))
                            nc.vector.tensor_tensor(
                                out=xs[:, tb, ch * 384:(ch + 1) * 384], in0=acc,
                                in1=x_b[b][:, tb, ch * 384:(ch + 1) * 384],
                                op=OP.add)
                    x2 = layer_norm(xs, work)
                    if taps and l == 0 and b == 0:
                        nc.sync.dma_start(tap["ctxT"], ctxT.bitcast(F32))
                        nc.sync.dma_start(tap["xs"], xs)
                        nc.sync.dma_start(tap["x2"], x2)

                    # FFN
                    x2T = transpose_b(x2)
                    xs2 = work.tile([P, TB, H], F32, tag="x")
                    for c in range(ICH):
                        wi_c = wif.tile([P, KO, 512], F32R, tag="wif")
                        nc.sync.dma_start(
                            wi_c, wi_d[l][:, c * 512:(c + 1) * 512]
                            .rearrange("(ko p) n -> p ko n", p=P).bitcast(F32R))
                        wf_c = wif.tile([P, KC, H], F32R, tag="wif")
                        nc.sync.dma_start(
                            wf_c, wf_d[l][c * 512:(c + 1) * 512, :]
                            .rearrange("(kc p) n -> p kc n", p=P).bitcast(F32R))
                        hT = hTp.tile([P, KC, S], F32R, tag="hT")
                        for icl in range(KC):
                            ps = mm.tile([P, 512], F32, tag="mm")
                            for k in range(KO):
                                nc.tensor.matmul(
                                    ps, lhsT=wi_c[:, k, icl * P:(icl + 1) * P],
                                    rhs=x2T[:, k, :],
                                    start=(k == 0), stop=(k == KO - 1))
                            nc.scalar.activation(hT[:, icl, :], ps, AF.Gelu)
                        if taps and l == 0 and b == 0 and c == 0:
                            nc.sync.dma_start(tap["hT0"], hT.bitcast(F32))
                        for tb in range(TB):
                            for ch in range(2):
                                acc = accp.tile([P, 384], F32, tag="acc")
                                for kc in range(KC):
                                    nc.tensor.matmul(
                                        acc,
                                        lhsT=hT[:, kc, tb * P:(tb + 1) * P],
                                        rhs=wf_c[:, kc, ch * 384:(ch + 1) * 384],
                                        start=(kc == 0), stop=(kc == KC - 1))
                                nc.vector.tensor_tensor(
                                    out=xs2[:, tb, ch * 384:(ch + 1) * 384],
                                    in0=acc,
                                    in1=(x2 if c == 0 else xs2)
                                        [:, tb, ch * 384:(ch + 1) * 384],
                                    op=OP.add)
                    if taps and l == 0 and b == 0:
                        nc.sync.dma_start(tap["xs2"], xs2)
                    x_b[b] = layer_norm(xs2, xres)

            # ---------------- outputs ----------------
            for b in range(BP):
                nc.sync.dma_start(
                    x_out[b * S:(b + 1) * S].rearrange("(t p) f -> p t f", p=P),
                    x_b[b])
                # bounce token-0 row to DRAM for the pooler transpose
                nc.sync.dma_start(bounce[b:b + 1, :], x_b[b][0:1, 0, :])

            # pooler: pooled = tanh(x[:,0] @ pool_W)
            xT0 = const.tile([P, KO, BP], F32R)
            for b in range(BP):
                nc.sync.dma_start(
                    xT0[:, :, b:b + 1],
                    bounce[b:b + 1, :].rearrange("o (k p) -> p k o", p=P).bitcast(F32R))
            pw_c = load_w4(pw_d)
            pool_sb = pl.tile([BP, H], F32, tag="pool")
            for ch in range(2):
                pool_ps = accp.tile([P, 384], F32, tag="acc")
                for k in range(KO):
                    nc.tensor.matmul(
                        pool_ps[0:BP, :],
                        lhsT=xT0[:, k, :], rhs=pw_c[ch][:, k, :],
                        start=(k == 0), stop=(k == KO - 1))
                nc.scalar.activation(pool_sb[:, ch * 384:(ch + 1) * 384],
                                     pool_ps[0:BP, :], AF.Tanh)
            nc.sync.dma_start(pooled_out, pool_sb)

    nc.compile()
    return nc


def _np_fallback(inputs):
    """Exact reference reimplementation on CPU via jax (only used when the
    device kernel's structural assumptions don't hold)."""
    import jax
    import jax.numpy as jnp
    with jax.default_device(jax.devices("cpu")[0]):
        ii = jnp.asarray(inputs["input_ids"])
        am = jnp.asarray(inputs["attention_mask"])
        tt = jnp.asarray(inputs["token_type_ids"])
        Bc, Sc = ii.shape
        dh = H // NH

        def _ln(x, g, b):
            m = x.mean(-1, keepdims=True)
            v = ((x - m) ** 2).mean(-1, keepdims=True)
            return (x - m) * jax.lax.rsqrt(v + EPS) * g + b

        star = ii >= OFF
        embs = jnp.where(
            star[..., None],
            jnp.asarray(inputs["star_emb"])[jnp.clip(ii - OFF, 0, STAR_V - 1)],
            jnp.asarray(inputs["word_emb"])[jnp.clip(ii, 0, VOCAB - 1)])
        x = _ln(embs + jnp.asarray(inputs["pos_emb"])[:Sc][None]
                + jnp.asarray(inputs["type_emb"])[tt],
                jnp.asarray(inputs["emb_g"]), jnp.asarray(inputs["emb_b"]))
        ext = (1.0 - am.astype(x.dtype))[:, None, None, :] * -10000.0
        scale = 1.0 / np.sqrt(dh).astype(np.float32)
        for l in range(L):
            wq, bq = inputs["Wq"][l], inputs["bq"][l]
            wk, bk = inputs["Wk"][l], inputs["bk"][l]
            wv, bv = inputs["Wv"][l], inputs["bv"][l]
            wo, bo = inputs["Wo"][l], inputs["bo"][l]
            sp = lambda t: t.reshape(Bc, Sc, NH, dh).transpose(0, 2, 1, 3)
            q, k, v = sp(x @ wq + bq), sp(x @ wk + bk), sp(x @ wv + bv)
            att = jax.nn.softmax(
                jnp.einsum("bhqd,bhkd->bhqk", q, k) * scale + ext, axis=-1)
            ctx = jnp.einsum("bhqk,bhkd->bhqd", att, v).transpose(0, 2, 1, 3)\
                .reshape(Bc, Sc, H)
            x = _ln(x + ctx @ wo + bo, inputs["g1"][l], inputs["b1"][l])
            hmid = jax.nn.gelu(x @ inputs["Wi"][l] + inputs["bi"][l],
                               approximate=False)
            x = _ln(x + hmid @ inputs["Wf"][l] + inputs["bf"][l],
                    inputs["g2"][l], inputs["b2"][l])
        pooled = jnp.tanh(x[:, 0] @ jnp.asarray(inputs["pool_W"])
                          + jnp.asarray(inputs["pool_b"]))
        return np.asarray(x), np.asarray(pooled)


def kernel(**inputs) -> np.ndarray:
    import concourse.bass_utils as bass_utils

    f32 = lambda k: np.ascontiguousarray(np.asarray(inputs[k], dtype=np.float32))

    # structural-triviality checks; anything unusual -> exact CPU fallback
    trivial = all(
        np.all(np.asarray(inputs[k]) == 0.0)
        for k in ("bq", "bk", "bv", "bo", "bi", "bf", "emb_b", "b1", "b2", "pool_b")
    ) and all(
        np.all(np.asarray(inputs[k]) == 1.0) for k in ("emb_g", "g1", "g2")
    )
    ids = np.asarray(inputs["input_ids"])
    am = np.asarray(inputs["attention_mask"])
    tt = np.asarray(inputs["token_type_ids"])
    if not trivial or ids.shape != (B, S):
        return _np_fallback(inputs)

    use_ext = not np.all(am == 1)
    n_layers = int(os.environ.get("KERNEL_NLAYERS", L))
    key = (use_ext, n_layers)
    if key not in _CACHE:
        _CACHE[key] = _build(use_ext, n_layers)
    nc = _CACHE[key]

    # host-side prep
    word = f32("word_emb")
    star = f32("star_emb")
    emb_tab = np.concatenate([word, star], axis=0)           # [31480, H]
    star_m = ids >= OFF
    comb = np.where(star_m,
                    VOCAB + np.clip(ids - OFF, 0, STAR_V - 1),
                    np.clip(ids, 0, VOCAB - 1)).astype(np.int32)  # [B, S]
    pos = f32("pos_emb")[:S]                                  # [S, H]
    type_vec = f32("type_emb")[tt.astype(np.int64)]           # [B, S, H]
    pt_full = pos[None] + type_vec                            # [B, S, H]
    ext_full = ((1.0 - am.astype(np.float32)) * -10000.0)     # [B, S]

    wq = f32("Wq"); wk = f32("Wk"); wv = f32("Wv"); wo = f32("Wo")
    wi = f32("Wi"); wf = f32("Wf"); pw = f32("pool_W")

    in_maps = []
    for c in range(NCORES):
        b0 = c * BP
        m = dict(
            emb=emb_tab,
            tok_idx=comb[b0:b0 + BP].reshape(NT * P, 1),
            pt=pt_full[b0:b0 + BP].reshape(BP * S, H),
            wq=wq, wk=wk, wv=wv, wo=wo, wi=wi, wf=wf, pw=pw,
        )
        if use_ext:
            m["ext"] = ext_full[b0:b0 + BP].reshape(BP * S, 1)
        in_maps.append(m)

    LAST["nc"] = nc
    LAST["in_maps"] = in_maps
    res = bass_utils.run_bass_kernel_spmd(nc, in_maps, core_ids=list(range(NCORES)))
    LAST["results"] = res

    x = np.concatenate(
        [r["x_out"].reshape(BP, S, H) for r in res.results], axis=0)
    pooled = np.concatenate([r["pooled_out"] for r in res.results], axis=0)
    return x, pooled


# revision 12
# speedup vs baseline: 9.9703x; 9.9703x over previous
"""Trainium2 Bass kernel for a 12-layer BERT-style encoder with star-token embedding
lookup (KVPLM). Full inputs in, full outputs out; data-parallel over batch across
8 NeuronCores (B=16 -> 2 per core), weights replicated.

Per-core design (per batch b of 2, tokens 512 = 4 blocks x 128 partitions):
  - activations kept SBUF-resident in "natural" [tok_p, feat] layout; a transposed
    f32r copy [feat_p, tok] is produced by PE transposes where matmuls need it
  - all matmuls run in float32r (fp32 with 11-bit mantissa) at full PE rate
  - attention computed with transposed scores S^T[j,i] = K^T.T @ Q^T so softmax
    normalization folds into a ones-column of the V stationary (denominator comes
    out as row 64 of the ctx PSUM) + a K=1 PE broadcast of the reciprocal
  - embedding gather via indirect DMA from a host-concatenated [word;star] table
  - LayerNorm via bn_stats/bn_aggr; GELU/exp/tanh on the scalar engine

kernel(**inputs) -> (x [16,512,768] f32, pooled [16,768] f32)
"""
import os
import numpy as np

# model dims (hardcoded per problem spec)
L, H, NH, F, S, VOCAB, STAR_V, OFF = 12, 768, 12, 3072, 512, 31090, 390, 30700
DH = H // NH              # 64
B = 16
NCORES = 8
BP = B // NCORES          # batches per core = 2
P = 128
TB = S // P               # token blocks per batch = 4
NT = BP * TB              # token tiles per core = 8
KO = H // P               # feature blocks = 6
ICH = F // 512            # FFN inter chunks of 512 = 6
KC = 512 // P             # k-blocks per chunk = 4
EPS = 1e-12
SCALE = 1.0 / 8.0         # 1/sqrt(64)

_CACHE: dict = {}
LAST: dict = {}


def _build(use_ext: bool, n_layers: int, taps: bool = False):
    import concourse.bass as bass
    import concourse.tile as tile
    import concourse.mybir as mybir
    from concourse import bacc
    from concourse.masks import make_identity

    F32, F32R, I32 = mybir.dt.float32, mybir.dt.float32r, mybir.dt.int32
    AF = mybir.ActivationFunctionType
    OP = mybir.AluOpType

    nc = bacc.Bacc("TRN2", target_bir_lowering=False, debug=False)

    # ---------------- DRAM I/O ----------------
    emb = nc.dram_tensor("emb", [VOCAB + STAR_V, H], F32, kind="ExternalInput").ap()
    tok_idx = nc.dram_tensor("tok_idx", [NT * P, 1], I32, kind="ExternalInput").ap()
    pt_dram = nc.dram_tensor("pt", [BP * S, H], F32, kind="ExternalInput").ap()
    wq_d = nc.dram_tensor("wq", [L, H, H], F32, kind="ExternalInput").ap()
    wk_d = nc.dram_tensor("wk", [L, H, H], F32, kind="ExternalInput").ap()
    wv_d = nc.dram_tensor("wv", [L, H, H], F32, kind="ExternalInput").ap()
    wo_d = nc.dram_tensor("wo", [L, H, H], F32, kind="ExternalInput").ap()
    wi_d = nc.dram_tensor("wi", [L, H, F], F32, kind="ExternalInput").ap()
    wf_d = nc.dram_tensor("wf", [L, F, H], F32, kind="ExternalInput").ap()
    pw_d = nc.dram_tensor("pw", [H, H], F32, kind="ExternalInput").ap()
    if use_ext:
        ext_d = nc.dram_tensor("ext", [BP * S, 1], F32, kind="ExternalInput").ap()

    x_out = nc.dram_tensor("x_out", [BP * S, H], F32, kind="ExternalOutput").ap()
    pooled_out = nc.dram_tensor("pooled_out", [BP, H], F32, kind="ExternalOutput").ap()
    bounce = nc.dram_tensor("bounce", [BP, H], F32).ap()  # internal scratch
    tap = {}
    if taps:
        for nm, shp in [("xT", [P, KO, S]), ("QT", [P, KO, S]), ("KT", [P, KO, S]),
                        ("vaug", [P, TB, NH, DH + 1]), ("e0", [P, TB, S]),
                        ("ctxT", [P, KO, S]), ("den0", [1, S]),
                        ("xs", [P, TB, H]), ("x2", [P, TB, H]),
                        ("hT0", [P, KC, S]), ("xs2", [P, TB, H])]:
            tap[nm] = nc.dram_tensor("tap_" + nm, shp, F32,
                                     kind="ExternalOutput").ap()

    with tile.TileContext(nc) as tc:
        with (
            tc.tile_pool(name="const", bufs=1) as const,
            tc.tile_pool(name="xres", bufs=3) as xres,
            tc.tile_pool(name="work", bufs=3) as work,
            tc.tile_pool(name="tT", bufs=4) as tT,
            tc.tile_pool(name="hTp", bufs=2) as hTp,
            tc.tile_pool(name="ep", bufs=2) as ep,
            tc.tile_pool(name="w4", bufs=2) as w4,
            tc.tile_pool(name="wif", bufs=2) as wif,
            tc.tile_pool(name="st", bufs=4) as st,
            tc.tile_pool(name="rb", bufs=2) as rb,
            tc.tile_pool(name="pl", bufs=1) as pl,
            tc.tile_pool(name="mm", bufs=3, space="PSUM") as mm,
            tc.tile_pool(name="ctxp", bufs=2, space="PSUM") as ctxp,
            tc.tile_pool(name="accp", bufs=3, space="PSUM") as accp,
        ):
            ident = const.tile([P, P], F32)
            make_identity(nc, ident)
            eps_sb = const.tile([P, 1], F32)
            nc.vector.memset(eps_sb, EPS)
            ones_f32 = const.tile([P, TB, NH, 1], F32)
            nc.vector.memset(ones_f32, 1.0)
            ones64_f32 = const.tile([1, DH], F32)
            nc.vector.memset(ones64_f32, 1.0)
            ones_col = const.tile([1, DH], F32R)
            nc.vector.tensor_copy(ones_col, ones64_f32)
            # V augmented with a ones column per head: [j_p, jb, head, dh+1]
            v_aug = const.tile([P, TB, NH, DH + 1], F32R)
            nc.vector.tensor_copy(v_aug[:, :, :, DH:DH + 1], ones_f32)
            idx_sb = const.tile([P, NT], I32)
            nc.sync.dma_start(idx_sb, tok_idx.rearrange("(t p) o -> p (t o)", p=P))
            if use_ext:
                ext_sb = const.tile([P, BP, TB, 1], F32)
                nc.sync.dma_start(
                    ext_sb, ext_d.rearrange("(b t p) o -> p b t (o)", p=P, b=BP))

            # ---------------- helpers ----------------
            def layer_norm(xin, outpool):
                """xin: tile [P, TB, H] f32 -> new tile from outpool, LN over H."""
                out = outpool.tile([P, TB, H], F32, tag="x")
                for tb in range(TB):
                    stats = st.tile([P, 3, 6], F32, tag="st")
                    for sg in range(3):
                        nc.vector.bn_stats(stats[:, sg, :],
                                           xin[:, tb, sg * 256:(sg + 1) * 256])
                    mv = st.tile([P, 2], F32, tag="mv")
                    nc.vector.bn_aggr(mv, stats)
                    std = st.tile([P, 1], F32, tag="sd")
                    nc.scalar.activation(std, mv[:, 1:2], AF.Sqrt, bias=eps_sb[:, :1])
                    rstd = st.tile([P, 1], F32, tag="rs")
                    nc.vector.reciprocal(rstd, std)
                    nc.vector.tensor_scalar(
                        out=out[:, tb, :], in0=xin[:, tb, :],
                        scalar1=mv[:, 0:1], scalar2=rstd[:, :1],
                        op0=OP.subtract, op1=OP.mult)
                return out

            def transpose_b(xin):
                """xin: tile [P, TB, H] f32 -> f32r [P(feat), KO, S] transposed."""
                xT = tT.tile([P, KO, S], F32R, tag="T")
                for fo in range(KO):
                    for tb in range(TB):
                        ps = mm.tile([P, 512], F32, tag="mm")
                        nc.tensor.transpose(ps[:, :P], xin[:, tb, fo * P:(fo + 1) * P],
                                            ident)
                        nc.scalar.copy(xT[:, fo, tb * P:(tb + 1) * P], ps[:, :P])
                return xT

            def load_w4(dram_l):  # [H, H] dram slice -> 2 chunk tiles [P, KO, 384] f32r
                tiles = []
                for ch in range(2):
                    t = w4.tile([P, KO, 384], F32R, tag="w4")
                    nc.sync.dma_start(
                        t, dram_l[:, ch * 384:(ch + 1) * 384]
                        .rearrange("(ko p) n -> p ko n", p=P).bitcast(F32R))
                    tiles.append(t)
                return tiles

            # ---------------- embedding ----------------
            x_b = []
            for b in range(BP):
                xe = work.tile([P, TB, H], F32, tag="x")
                for tb in range(TB):
                    nc.gpsimd.indirect_dma_start(
                        out=xe[:, tb, :], out_offset=None, in_=emb,
                        in_offset=bass.IndirectOffsetOnAxis(
                            ap=idx_sb[:, b * TB + tb:b * TB + tb + 1], axis=0))
                ptb = work.tile([P, TB, H], F32, tag="x")
                nc.sync.dma_start(
                    ptb, pt_dram[b * S:(b + 1) * S].rearrange("(t p) f -> p t f", p=P))
                xs = work.tile([P, TB, H], F32, tag="x")
                nc.vector.tensor_tensor(out=xs, in0=xe, in1=ptb, op=OP.add)
                x_b.append(layer_norm(xs, xres))

            # ---------------- encoder layers ----------------
            for l in range(n_layers):
                for b in range(BP):
                    xT = transpose_b(x_b[b])

                    # Q^T, K^T: [feat_out_p, KO, S]; stationary = W chunks
                    qkT = []
                    for w_dram in (wq_d, wk_d):
                        w_c = load_w4(w_dram[l % L])
                        t = tT.tile([P, KO, S], F32R, tag="T")
                        for ch in range(2):
                            for hl in range(3):
                                ps = mm.tile([P, 512], F32, tag="mm")
                                for k in range(KO):
                                    nc.tensor.matmul(
                                        ps, lhsT=w_c[ch][:, k, hl * P:(hl + 1) * P],
                                        rhs=xT[:, k, :],
                                        start=(k == 0), stop=(k == KO - 1))
                                nc.vector.tensor_copy(t[:, ch * 3 + hl, :], ps)
                        qkT.append(t)
                    QT, KT = qkT
                    if taps and l == 0 and b == 0:
                        nc.sync.dma_start(tap["xT"], xT.bitcast(F32))
                        nc.sync.dma_start(tap["QT"], QT.bitcast(F32))
                        nc.sync.dma_start(tap["KT"], KT.bitcast(F32))

                    # V natural into v_aug[:, tb, h, :DH]
                    wv_c = load_w4(wv_d[l % L])
                    for ch in range(2):
                        for tb in range(TB):
                            ps = mm.tile([P, 512], F32, tag="mm")
                            for k in range(KO):
                                nc.tensor.matmul(
                                    ps[:, :384], lhsT=xT[:, k, tb * P:(tb + 1) * P],
                                    rhs=wv_c[ch][:, k, :],
                                    start=(k == 0), stop=(k == KO - 1))
                            nc.vector.tensor_copy(
                                v_aug[:, tb, ch * 6:(ch + 1) * 6, 0:DH],
                                ps[:, :384].rearrange("p (h d) -> p h d", d=DH))

                    if taps and l == 0 and b == 0:
                        nc.sync.dma_start(tap["vaug"], v_aug.bitcast(F32))

                    # attention per head
                    ctxT = tT.tile([P, KO, S], F32R, tag="T")
                    for h in range(NH):
                        pb = DH * (h % 2)
                        fo = h // 2
                        qT_h = QT[pb:pb + DH, fo, :]
                        ctx_ps = ctxp.tile([P, 512], F32, tag="ctx")
                        for jb in range(TB):
                            s_ps = mm.tile([P, 512], F32, tag="mm")
                            nc.tensor.matmul(
                                s_ps, lhsT=KT[pb:pb + DH, fo, jb * P:(jb + 1) * P],
                                rhs=qT_h, start=True, stop=True)
                            e_t = ep.tile([P, 512], F32R, tag="e")
                            bias = ext_sb[:, b, jb, :1] if use_ext else 0.0
                            nc.scalar.activation(e_t, s_ps, AF.Exp,
                                                 scale=SCALE, bias=bias)
                            if taps and l == 0 and b == 0 and h == 0:
                                nc.sync.dma_start(tap["e0"][:, jb, :],
                                                  e_t.bitcast(F32))
                            nc.tensor.matmul(
                                ctx_ps[0:DH + 1, :], lhsT=v_aug[:, jb, h, :],
                                rhs=e_t, start=(jb == 0), stop=(jb == TB - 1))
                        # normalize: ctxT[head rows] = ctx * (1/denom) broadcast
                        rec = rb.tile([1, 512], F32R, tag="rec")
                        with nc.allow_low_precision(reason="f32r recip"):
                            nc.vector.reciprocal(rec, ctx_ps[DH:DH + 1, :])
                        bc_ps = ctxp.tile([P, 512], F32, tag="ctx")
                        nc.tensor.matmul(bc_ps[0:DH, :], lhsT=ones_col, rhs=rec,
                                         start=True, stop=True)
                        bc_sb = rb.tile([DH, 512], F32, tag="bc")
                        nc.vector.tensor_copy(bc_sb, bc_ps[0:DH, :])
                        if taps and l == 0 and b == 0 and h == 0:
                            nc.sync.dma_start(tap["den0"], rec.bitcast(F32))
                        nc.vector.tensor_tensor(
                            out=ctxT[pb:pb + DH, fo, :], in0=ctx_ps[0:DH, :],
                            in1=bc_sb, op=OP.mult)

                    # O projection + residual
                    wo_c = load_w4(wo_d[l % L])
                    xs = work.tile([P, TB, H], F32, tag="x")
                    for tb in range(TB):
                        for ch in range(2):
                            acc = accp.tile([P, 384], F32, tag="acc")
                            for k in range(KO):
                                nc.tensor.matmul(
                                    acc,
                                    lhsT=ctxT[:, k, tb * P:(tb + 1) * P],
                                    rhs=wo_c[ch][:, k, :],
                                    start=(k == 0), stop=(k == KO - 1))
                            nc.vector.tensor_tensor(
                                out=xs[:, tb, ch * 384:(ch + 1) * 384], in0=acc,
                                in1=x_b[b][:, tb, ch * 384:(ch + 1) * 384],
                                op=OP.add)
                    x2 = layer_norm(xs, work)
                    if taps and l == 0 and b == 0:
                        nc.sync.dma_start(tap["ctxT"], ctxT.bitcast(F32))
                        nc.sync.dma_start(tap["xs"], xs)
                        nc.sync.dma_start(tap["x2"], x2)

                    # FFN
                    x2T = transpose_b(x2)
                    xs2 = work.tile([P, TB, H], F32, tag="x")
                    for c in range(ICH):
                        wi_c = wif.tile([P, KO, 512], F32R, tag="wif")
                        nc.sync.dma_start(
                            wi_c, wi_d[l % L][:, c * 512:(c + 1) * 512]
                            .rearrange("(ko p) n -> p ko n", p=P).bitcast(F32R))
                        wf_c = wif.tile([P, KC, H], F32R, tag="wif")
                        nc.sync.dma_start(
                            wf_c, wf_d[l % L][c * 512:(c + 1) * 512, :]
                            .rearrange("(kc p) n -> p kc n", p=P).bitcast(F32R))
                        hT = hTp.tile([P, KC, S], F32R, tag="hT")
                        for icl in range(KC):
                            ps = mm.tile([P, 512], F32, tag="mm")
                            for k in range(KO):
                                nc.tensor.matmul(
                                    ps, lhsT=wi_c[:, k, icl * P:(icl + 1) * P],
                                    rhs=x2T[:, k, :],
                                    start=(k == 0), stop=(k == KO - 1))
                            nc.scalar.activation(hT[:, icl, :], ps, AF.Gelu)
                        if taps and l == 0 and b == 0 and c == 0:
                            nc.sync.dma_start(tap["hT0"], hT.bitcast(F32))
                        for tb in range(TB):
                            for ch in range(2):
                                acc = accp.tile([P, 384], F32, tag="acc")
                                for kc in range(KC):
                                    nc.tensor.matmul(
                                        acc,
                                        lhsT=hT[:, kc, tb * P:(tb + 1) * P],
                                        rhs=wf_c[:, kc, ch * 384:(ch + 1) * 384],
                                        start=(kc == 0), stop=(kc == KC - 1))
                                nc.vector.tensor_tensor(
                                    out=xs2[:, tb, ch * 384:(ch + 1) * 384],
                                    in0=acc,
                                    in1=(x2 if c == 0 else xs2)
                                        [:, tb, ch * 384:(ch + 1) * 384],
                                    op=OP.add)
                    if taps and l == 0 and b == 0:
                        nc.sync.dma_start(tap["xs2"], xs2)
                    x_b[b] = layer_norm(xs2, xres)

            # ---------------- outputs ----------------
            for b in range(BP):
                nc.sync.dma_start(
                    x_out[b * S:(b + 1) * S].rearrange("(t p) f -> p t f", p=P),
                    x_b[b])
                # bounce token-0 row to DRAM for the pooler transpose
                nc.sync.dma_start(bounce[b:b + 1, :], x_b[b][0:1, 0, :])

            # pooler: pooled = tanh(x[:,0] @ pool_W)
            xT0 = const.tile([P, KO, BP], F32R)
            for b in range(BP):
                nc.sync.dma_start(
                    xT0[:, :, b:b + 1],
                    bounce[b:b + 1, :].rearrange("o (k p) -> p k o", p=P).bitcast(F32R))
            pw_c = load_w4(pw_d)
            pool_sb = pl.tile([BP, H], F32, tag="pool")
            for ch in range(2):
                pool_ps = accp.tile([P, 384], F32, tag="acc")
                for k in range(KO):
                    nc.tensor.matmul(
                        pool_ps[0:BP, :],
                        lhsT=xT0[:, k, :], rhs=pw_c[ch][:, k, :],
                        start=(k == 0), stop=(k == KO - 1))
                nc.scalar.activation(pool_sb[:, ch * 384:(ch + 1) * 384],
                                     pool_ps[0:BP, :], AF.Tanh)
            nc.sync.dma_start(pooled_out, pool_sb)

    nc.compile()
    return nc


def _np_fallback(inputs):
    """Exact reference reimplementation on CPU via jax (only used when the
    device kernel's structural assumptions don't hold)."""
    import jax
    import jax.numpy as jnp
    with jax.default_device(jax.devices("cpu")[0]):
        ii = jnp.asarray(inputs["input_ids"])
        am = jnp.asarray(inputs["attention_mask"])
        tt = jnp.asarray(inputs["token_type_ids"])
        Bc, Sc = ii.shape
        dh = H // NH

        def _ln(x, g, b):
            m = x.mean(-1, keepdims=True)
            v = ((x - m) ** 2).mean(-1, keepdims=True)
            return (x - m) * jax.lax.rsqrt(v + EPS) * g + b

        star = ii >= OFF
        embs = jnp.where(
            star[..., None],
            jnp.asarray(inputs["star_emb"])[jnp.clip(ii - OFF, 0, STAR_V - 1)],
            jnp.asarray(inputs["word_emb"])[jnp.clip(ii, 0, VOCAB - 1)])
        x = _ln(embs + jnp.asarray(inputs["pos_emb"])[:Sc][None]
                + jnp.asarray(inputs["type_emb"])[tt],
                jnp.asarray(inputs["emb_g"]), jnp.asarray(inputs["emb_b"]))
        ext = (1.0 - am.astype(x.dtype))[:, None, None, :] * -10000.0
        scale = 1.0 / np.sqrt(dh).astype(np.float32)
        for l in range(L):
            wq, bq = inputs["Wq"][l], inputs["bq"][l]
            wk, bk = inputs["Wk"][l], inputs["bk"][l]
            wv, bv = inputs["Wv"][l], inputs["bv"][l]
            wo, bo = inputs["Wo"][l], inputs["bo"][l]
            sp = lambda t: t.reshape(Bc, Sc, NH, dh).transpose(0, 2, 1, 3)
            q, k, v = sp(x @ wq + bq), sp(x @ wk + bk), sp(x @ wv + bv)
            att = jax.nn.softmax(
                jnp.einsum("bhqd,bhkd->bhqk", q, k) * scale + ext, axis=-1)
            ctx = jnp.einsum("bhqk,bhkd->bhqd", att, v).transpose(0, 2, 1, 3)\
                .reshape(Bc, Sc, H)
            x = _ln(x + ctx @ wo + bo, inputs["g1"][l], inputs["b1"][l])
            hmid = jax.nn.gelu(x @ inputs["Wi"][l] + inputs["bi"][l],
                               approximate=False)
            x = _ln(x + hmid @ inputs["Wf"][l] + inputs["bf"][l],
                    inputs["g2"][l], inputs["b2"][l])
        pooled = jnp.tanh(x[:, 0] @ jnp.asarray(inputs["pool_W"])
                          + jnp.asarray(inputs["pool_b"]))
        return np.asarray(x), np.asarray(pooled)


def kernel(**inputs) -> np.ndarray:
    import concourse.bass_utils as bass_utils

    f32 = lambda k: np.ascontiguousarray(np.asarray(inputs[k], dtype=np.float32))

    # structural-triviality checks; anything unusual -> exact CPU fallback
    trivial = all(
        np.all(np.asarray(inputs[k]) == 0.0)
        for k in ("bq", "bk", "bv", "bo", "bi", "bf", "emb_b", "b1", "b2", "pool_b")
    ) and all(
        np.all(np.asarray(inputs[k]) == 1.0) for k in ("emb_g", "g1", "g2")
    )
    ids = np.asarray(inputs["input_ids"])
    am = np.asarray(inputs["attention_mask"])
    tt = np.asarray(inputs["token_type_ids"])
    if not trivial or ids.shape != (B, S):
        return _np_fallback(inputs)

    use_ext = not np.all(am == 1)
    n_layers = int(os.environ.get("KERNEL_NLAYERS", L))
    key = (use_ext, n_layers)
    if key not in _CACHE:
        _CACHE[key] = _build(use_ext, n_layers)
    nc = _CACHE[key]

    # host-side prep
    word = f32("word_emb")
    star = f32("star_emb")
    emb_tab = np.concatenate([word, star], axis=0)           # [31480, H]
    star_m = ids >= OFF
    comb = np.where(star_m,
                    VOCAB + np.clip(ids - OFF, 0, STAR_V - 1),
                    np.clip(ids, 0, VOCAB - 1)).astype(np.int32)  # [B, S]
    pos = f32("pos_emb")[:S]                                  # [S, H]
    type_vec = f32("type_emb")[tt.astype(np.int64)]           # [B, S, H]
    pt_full = pos[None] + type_vec                            # [B, S, H]
    ext_full = ((1.0 - am.astype(np.float32)) * -10000.0)     # [B, S]

    wq = f32("Wq"); wk = f32("Wk"); wv = f32("Wv"); wo = f32("Wo")
    wi = f32("Wi"); wf = f32("Wf"); pw = f32("pool_W")

    in_maps = []
    for c in range(NCORES):
        b0 = c * BP
        m = dict(
            emb=emb_tab,
            tok_idx=comb[b0:b0 + BP].reshape(NT * P, 1),
            pt=pt_full[b0:b0 + BP].reshape(BP * S, H),
            wq=wq, wk=wk, wv=wv, wo=wo, wi=wi, wf=wf, pw=pw,
        )
        if use_ext:
            m["ext"] = ext_full[b0:b0 + BP].reshape(BP * S, 1)
        in_maps.append(m)

    LAST["nc"] = nc
    LAST["in_maps"] = in_maps
    try:
        res = bass_utils.run_bass_kernel_spmd(
            nc, in_maps, core_ids=list(range(NCORES)))
    except Exception:
        # transient NRT_EXEC_UNIT_UNRECOVERABLE after device churn: retry once
        res = bass_utils.run_bass_kernel_spmd(
            nc, in_maps, core_ids=list(range(NCORES)))
    LAST["results"] = res

    x = np.concatenate(
        [r["x_out"].reshape(BP, S, H) for r in res.results], axis=0)
    pooled = np.concatenate([r["pooled_out"] for r in res.results], axis=0)
    return x, pooled
